# revision 3
# baseline (speedup 1.0000x reference)
"""Multi-head attention on 8 Trainium2 NeuronCores — batch x head sharding.

Core i handles batch b=i//2 and head-group g=i%2 (8 heads, 512 proj dims).
Host sums the 2 partial outputs per batch and adds b_o + b_v @ w_o.

Key structure vs the old head-only kernel:
- Scores pre-scaled by F = 128*log2(e): folded sqrt(F) into w_q AND w_k, so
  exp(s/8) = 2^(s'/1024), enabling a 1-op Schraudolph exp on the DVE:
  int16(s' + 15360) bit-cast as fp16. Half-ish of the exps run there, the
  rest as exact ACT exp with scale 1/(8F) — splitting the softmax exp load
  across two engines (it would otherwise be the serial bottleneck).
- Score matmuls row-tile-paired: head-even in PE rows 0-63, head-odd in rows
  64-127, concurrent -> full 128-row PE utilization despite dh=64.
- AV keeps the [V|1] ones-column trick per head for softmax denominators;
  both av accumulators live at base partition 0 in separate banks.
- 1/r via reciprocal_approx_fast + gpsimd partition_broadcast; the PSUM->SBUF
  attnT copy is fused with the 1/r scaling (tensor_mul, output-shifted for
  the odd head).
- Output projection accumulates all 4 head-pairs in PSUM (full K=128 MMs),
  single copy out per chunk.
"""

import numpy as np

import concourse.bacc as bacc
import concourse.mybir as mybir
from concourse.tile import TileContext
from concourse import bass_utils

dt = mybir.dt
F32 = dt.float32
F16 = dt.float16
I16 = dt.int16
AF = mybir.ActivationFunctionType
ALU = mybir.AluOpType

B, S, D = 4, 2048, 1024
H, DH = 16, 64
NCORES = 8
HC = 8                      # heads per core
GW = HC * DH                # 512 proj cols per core
NP = HC // 2                # head pairs = 4

FSC = 128.0 * np.log2(np.e)          # score pre-scale (folded into wq,wk)
EXP_SCALE = float(1.0 / (8.0 * FSC))  # ACT exp scale on pre-scaled scores
BCONST = 15360.0                      # schraudolph bias (fp16 exponent 15)

_CACHE = {}


def build_nc(s=S, d=D):
    n_kt = d // 128            # 8 contraction tiles
    n_tt = s // 128            # 16 token tiles
    n_jc = s // 512            # 4 query chunks
    nc = bacc.Bacc("TRN2", target_bir_lowering=False, debug=False)

    x_d = nc.dram_tensor("x", [d, s], F16, kind="ExternalInput")
    wq_d = nc.dram_tensor("wq", [d, GW], F16, kind="ExternalInput")
    wk_d = nc.dram_tensor("wk", [d, GW], F16, kind="ExternalInput")
    wv_d = nc.dram_tensor("wv", [d, GW], F16, kind="ExternalInput")
    bq_d = nc.dram_tensor("bq", [128, 4], F32, kind="ExternalInput")
    bk_d = nc.dram_tensor("bk", [128, 4], F32, kind="ExternalInput")
    wo_d = nc.dram_tensor("wo", [GW, d], F16, kind="ExternalInput")
    out_d = nc.dram_tensor("out", [s, d], F16, kind="ExternalOutput")

    with TileContext(nc) as tc:
        with (
            tc.tile_pool(name="const", bufs=1) as cpool,
            tc.tile_pool(name="wpool", bufs=1) as wpool,
            tc.tile_pool(name="xt", bufs=n_kt) as xt_pool,
            tc.tile_pool(name="qk", bufs=2 * NP) as qk_pool,
            tc.tile_pool(name="vt", bufs=1) as vt_pool,
            tc.tile_pool(name="at", bufs=NP) as at_pool,
            tc.tile_pool(name="pex", bufs=6) as pex_pool,
            tc.tile_pool(name="rl", bufs=2) as rl_pool,
            tc.tile_pool(name="osb", bufs=2) as osb_pool,
            tc.tile_pool(name="ps", bufs=1, space="PSUM") as pp,
        ):
            # ---- loads: first half of x + wq first, so proj starts ASAP ----
            xT = [None] * n_kt
            w16 = {}

            def load_x(kts):
                for kt in kts:
                    xt = xt_pool.tile([128, s], F16, tag="xt", name=f"xT{kt}")
                    nc.sync.dma_start(
                        out=xt[:, :], in_=x_d[kt * 128:(kt + 1) * 128, :])
                    xT[kt] = xt

            def load_w(name, dram):
                for kt in range(n_kt):
                    wt = wpool.tile([128, GW], F16, tag="w", bufs=3 * n_kt,
                                    name=f"w_{name}{kt}")
                    nc.sync.dma_start(
                        out=wt[:, :], in_=dram[kt * 128:(kt + 1) * 128, :])
                    w16[(name, kt)] = wt

            load_x(range(4))
            load_w("q", wq_d)
            load_x(range(4, n_kt))
            load_w("k", wk_d)
            load_w("v", wv_d)

            bqt = cpool.tile([128, 4], F32, tag="bqt")
            bkt = cpool.tile([128, 4], F32, tag="bkt")
            nc.sync.dma_start(out=bqt[:, :], in_=bq_d[:, :])
            nc.sync.dma_start(out=bkt[:, :], in_=bk_d[:, :])

            wo_sb = []
            for hp in range(NP):
                wt = wpool.tile([128, d], F16, tag="wo", bufs=NP, name=f"wo{hp}")
                nc.sync.dma_start(
                    out=wt[:, :], in_=wo_d[hp * 128:(hp + 1) * 128, :])
                wo_sb.append(wt)

            # vt: [128 tok, (hp, tt, two, 65)] fp16, ones at col 64
            vt = vt_pool.tile([128, NP * n_tt * 130 + 128], F16, tag="vt")
            nc.vector.memset(vt[:, NP * n_tt * 130:], 0.0)
            vt4 = vt[:, 0:NP * n_tt * 130].rearrange(
                "p (hp t two c) -> p hp t two c", hp=NP, t=n_tt, two=2, c=65)
            nc.vector.memset(vt4[:, :, :, :, 64:65], 1.0)

            qT = [qk_pool.tile([128, s], F16, tag="qz", bufs=2 * NP,
                               name=f"qT{h}") for h in range(2 * NP)]
            kT = [qk_pool.tile([128, s], F16, tag="qk", name=f"kT{hp}")
                  for hp in range(NP)]
            for h in range(2 * NP):
                if h % 2 == 0:
                    nc.vector.memset(qT[h][64:128, :], 0.0)
                else:
                    nc.vector.memset(qT[h][0:64, :], 0.0)
            attnT = [at_pool.tile([128, s], F16, tag="at", name=f"aT{hp}")
                     for hp in range(NP)]

            # ---- Q/K projections (hp 0 first so attention can start) ----
            def proj_qk(hp):
                for name, bias, dst in (("q", bqt, None), ("k", bkt, kT[hp])):
                    p1 = pp.tile([128, 512], F32, tag="sta", bufs=2,
                                 name=f"pj1_{name}{hp}")
                    p2 = pp.tile([128, 512], F32, tag="stb", bufs=2,
                                 name=f"pj2_{name}{hp}")
                    p3 = pp.tile([128, 512], F32, tag="ps1", bufs=4,
                                 name=f"pj3_{name}{hp}")
                    p4 = pp.tile([128, 512], F32, tag="ps1", bufs=4,
                                 name=f"pj4_{name}{hp}")
                    accs = [p1[:, :], p2[:, :], p3[:, :], p4[:, :]]
                    for kt in range(n_kt):
                        lh = w16[(name, kt)][:, hp * 128:(hp + 1) * 128]
                        for tcn in range(4):
                            nc.tensor.matmul(
                                accs[tcn], lh,
                                xT[kt][:, tcn * 512:(tcn + 1) * 512],
                                start=(kt == 0), stop=(kt == n_kt - 1),
                            )
                    for tcn in range(4):
                        cs = slice(tcn * 512, (tcn + 1) * 512)
                        if name == "k":
                            nc.vector.tensor_scalar_add(
                                dst[:, cs], accs[tcn], bias[:, hp:hp + 1])
                        else:
                            nc.vector.tensor_scalar_add(
                                qT[2 * hp][0:64, cs], accs[tcn][0:64, :],
                                bias[0:64, hp:hp + 1])
                            nc.vector.tensor_scalar_add(
                                qT[2 * hp + 1][64:128, cs],
                                accs[tcn][64:128, :], bias[64:128, hp:hp + 1])

            proj_qk(0)

            # ---- V projection (natural layout, interleaved into vt) ----
            for tt in range(n_tt):
                pv = pp.tile([128, 512], F32, tag="ps1", bufs=4, name=f"pv{tt}")
                for kt in range(n_kt):
                    nc.tensor.matmul(
                        pv[:, :], xT[kt][:, tt * 128:(tt + 1) * 128],
                        w16[("v", kt)][:, :],
                        start=(kt == 0), stop=(kt == n_kt - 1),
                    )
                pv4 = pv.rearrange("p (hp two c) -> p hp two c",
                                   hp=NP, two=2, c=64)
                nc.vector.tensor_copy(vt4[:, :, tt, 0, 0:64], pv4[:, :, 0, :])
                nc.vector.tensor_copy(vt4[:, :, tt, 1, 0:64], pv4[:, :, 1, :])

            # ---- attention (proj of next pair interleaved between windows) ----
            for hp in range(NP):
                if hp + 1 < NP:
                    proj_qk(hp + 1)
                for jc in range(n_jc):
                    qs = slice(jc * 512, (jc + 1) * 512)
                    av0 = pp.tile([128, 512], F32, tag="ps1", bufs=4,
                                  name=f"av0_{hp}_{jc}")
                    av1 = pp.tile([128, 512], F32, tag="ps1", bufs=4,
                                  name=f"av1_{hp}_{jc}")
                    for kt in range(n_tt):
                        ks = slice(kt * 128, (kt + 1) * 128)
                        st0 = pp.tile([128, 512], F32, tag="sta", bufs=2,
                                      name=f"st0_{hp}_{jc}_{kt}")
                        st1 = pp.tile([128, 512], F32, tag="stb", bufs=2,
                                      name=f"st1_{hp}_{jc}_{kt}")
                        nc.tensor.matmul(
                            st0[:, :], kT[hp][:, ks], qT[2 * hp][:, qs],
                            start=True, stop=True,
                        )
                        nc.tensor.matmul(
                            st1[:, :], kT[hp][:, ks],
                            qT[2 * hp + 1][:, qs],
                            start=True, stop=True,
                        )
                        pex0 = pex_pool.tile([128, 512], F16, tag="pexa",
                                             bufs=4, name=f"pex0_{hp}_{jc}_{kt}")
                        pex1 = pex_pool.tile([128, 512], F16, tag="pexb",
                                             bufs=4, name=f"pex1_{hp}_{jc}_{kt}")
                        nc.scalar.activation(
                            pex0[:, :], st0[:, :], AF.Exp, scale=EXP_SCALE)
                        nc.vector.tensor_scalar_add(
                            pex1[:, :].bitcast(I16), st1[:, :], BCONST)
                        vbase = (hp * n_tt + kt) * 130
                        nc.tensor.matmul(
                            av0[:, :], vt[:, vbase:vbase + 128],
                            pex0[:, :],
                            start=(kt == 0), stop=(kt == n_tt - 1),
                        )
                        nc.tensor.matmul(
                            av1[:, :], vt[:, vbase + 65:vbase + 193],
                            pex1[:, :],
                            start=(kt == 0), stop=(kt == n_tt - 1),
                        )
                    # drain: r rows -> 1/r -> broadcast -> scaled attnT copy
                    rline = rl_pool.tile([1, 1024], F32, tag="rl",
                                         name=f"rline_{hp}_{jc}")
                    rrec = rl_pool.tile([1, 1024], F32, tag="rr",
                                        name=f"rrec_{hp}_{jc}")
                    rb = rl_pool.tile([64, 1024], F32, tag="rb",
                                      name=f"rb_{hp}_{jc}")
                    nc.scalar.copy(rline[0:1, 0:512], av0[64:65, :])
                    nc.scalar.copy(rline[0:1, 512:1024], av1[64:65, :])
                    nc.vector.reciprocal_approx_fast(rrec[0:1, :],
                                                     rline[0:1, :])
                    nc.gpsimd.partition_broadcast(rb[:, :], rrec[0:1, :])
                    nc.vector.tensor_mul(attnT[hp][0:64, qs], av0[0:64, :],
                                         rb[:, 0:512])
                    nc.vector.tensor_mul(attnT[hp][64:128, qs], av1[0:64, :],
                                         rb[:, 512:1024])

            # ---- output projection: accumulate 4 pairs in PSUM ----
            for tt in range(n_tt):
                ts_ = slice(tt * 128, (tt + 1) * 128)
                po0 = pp.tile([128, 512], F32, tag="ps1", bufs=4, name=f"po0_{tt}")
                po1 = pp.tile([128, 512], F32, tag="ps1", bufs=4, name=f"po1_{tt}")
                for hp in range(NP):
                    nc.tensor.matmul(
                        po0[:, :], attnT[hp][:, ts_], wo_sb[hp][:, 0:512],
                        start=(hp == 0), stop=(hp == NP - 1),
                    )
                    nc.tensor.matmul(
                        po1[:, :], attnT[hp][:, ts_], wo_sb[hp][:, 512:1024],
                        start=(hp == 0), stop=(hp == NP - 1),
                    )
                osb = osb_pool.tile([128, 1024], F16, tag="osb",
                                    name=f"osb{tt}")
                nc.vector.tensor_copy(osb[:, 0:512], po0[:, :])
                nc.scalar.copy(osb[:, 512:1024], po1[:, :])
                nc.sync.dma_start(out=out_d[ts_, :], in_=osb[:, :])

    nc.compile()
    return nc


def _get_nc():
    if "nc" not in _CACHE:
        _CACHE["nc"] = build_nc()
    return _CACHE["nc"]


def make_in_maps(x, w_q, b_q, w_k, b_k, w_v, w_o):
    sF = np.float32(np.sqrt(FSC))
    xT16 = [np.ascontiguousarray(np.asarray(x[b], np.float16).T)
            for b in range(x.shape[0])]
    wq16 = np.asarray(np.asarray(w_q, np.float32) * sF, np.float16)
    wk16 = np.asarray(np.asarray(w_k, np.float32) * sF, np.float16)
    wv16 = np.asarray(w_v, np.float16)
    wo16 = np.asarray(w_o, np.float16)
    bq32 = np.asarray(b_q, np.float32) * sF
    bk32 = np.asarray(b_k, np.float32) * sF

    in_maps = []
    for i in range(NCORES):
        b, g = i // 2, i % 2
        gs = slice(g * GW, (g + 1) * GW)
        in_maps.append({
            "x": xT16[b],
            "wq": np.ascontiguousarray(wq16[:, gs]),
            "wk": np.ascontiguousarray(wk16[:, gs]),
            "wv": np.ascontiguousarray(wv16[:, gs]),
            "bq": np.ascontiguousarray(bq32[gs].reshape(4, 128).T),
            "bk": np.ascontiguousarray(bk32[gs].reshape(4, 128).T),
            "wo": np.ascontiguousarray(wo16[gs, :]),
        })
    return in_maps


def kernel(x, w_q, b_q, w_k, b_k, w_v, b_v, w_o, b_o, _trace=False):
    x = np.asarray(x, dtype=np.float32)
    nc = _get_nc()
    in_maps = make_in_maps(x, w_q, b_q, w_k, b_k, w_v, w_o)
    kw = {}
    if _trace:
        import tempfile
        kw = dict(trace=True, trace_cores=list(range(NCORES)),
                  tmpdir=tempfile.mkdtemp(prefix="mha_trace_"))
    res = bass_utils.run_bass_kernel_spmd(
        nc, in_maps, core_ids=list(range(NCORES)), **kw
    )
    out = np.zeros(x.shape, dtype=np.float32)
    for b in range(x.shape[0]):
        out[b] = (np.asarray(res.results[2 * b]["out"], dtype=np.float32)
                  + np.asarray(res.results[2 * b + 1]["out"],
                               dtype=np.float32))
    out += np.asarray(b_o, dtype=np.float32)[None, None, :]
    out += (np.asarray(b_v, dtype=np.float32)
            @ np.asarray(w_o, dtype=np.float32))[None, None, :]
    if _trace:
        return out, res
    return out


# revision 4
# speedup vs baseline: 1.0027x; 1.0027x over previous
"""Multi-head attention on 8 Trainium2 NeuronCores — batch x head sharding.

Core i handles batch b=i//2 and head-group g=i%2 (8 heads, 512 proj dims).
Host sums the 2 partial outputs per batch and adds b_o + b_v @ w_o
(sum_k softmax = 1 makes the v-bias contribution exactly b_v @ w_o).

Design notes (what made this fast — 550us baseline -> ~400us):
- Every matmul is full-K=128 with a 128-column fp16 lhsT and no
  tile_position. In that shape walrus/PE hide LDWEIGHTS completely
  (216 ns/MM sustained); row-tiled tile_position pairs do NOT overlap
  under this toolchain and their weight loads serialize, measured slower.
- Scores: lhsT = kT pair tile (shared by both heads -> weight reuse);
  the per-head moving operand qT is zero-padded ([q_h; 0] / [0; q_h]) so
  the other head's rows contribute nothing.
- AV: vt layout [V_h0 | 1 | V_h1 | 1] per 130 cols; lhsT slices are
  widened to 128 cols so both heads read [V_h | 1 | junk]; the junk rows
  land in unused PSUM partitions, the ones-column yields the softmax
  denominator r at psum row 64 for both heads.
- Softmax exp is split across two engines per kt: head-even as exact ACT
  exp, head-odd as a one-op Schraudolph on the DVE: scores are pre-scaled
  by F = 128*log2(e) (folded sqrt(F) into w_q AND w_k), so
  exp(s/8) = 2^(s'/1024) = bitcast_fp16(int16(s' + 15360)).
- 1/r via reciprocal_approx_fast + gpsimd partition_broadcast; the
  PSUM->SBUF attnT copy is fused with the 1/r scaling (tensor_mul with a
  partition-shifted output for the odd head).
- Output projection accumulates all 4 head-pairs in PSUM, one copy out.
- Fine-grained single-bank PSUM tiles (per-head st/pexp) decouple the two
  exp engine chains; keeping PE duty >85 percent also keeps the HAM clock
  gate at 2.4 GHz (coarse tiles measured 30 percent slower from K=4/8
  throttling).
"""

import numpy as np

import concourse.bacc as bacc
import concourse.mybir as mybir
from concourse.tile import TileContext
from concourse import bass_utils

dt = mybir.dt
F32 = dt.float32
F16 = dt.float16
I16 = dt.int16
AF = mybir.ActivationFunctionType
ALU = mybir.AluOpType

B, S, D = 4, 2048, 1024
H, DH = 16, 64
NCORES = 8
HC = 8                      # heads per core
GW = HC * DH                # 512 proj cols per core
NP = HC // 2                # head pairs = 4

FSC = 128.0 * np.log2(np.e)          # score pre-scale (folded into wq,wk)
EXP_SCALE = float(1.0 / (8.0 * FSC))  # ACT exp scale on pre-scaled scores
BCONST = 15360.0                      # schraudolph bias (fp16 exponent 15)

_CACHE = {}


def build_nc(s=S, d=D):
    n_kt = d // 128            # 8 contraction tiles
    n_tt = s // 128            # 16 token tiles
    n_jc = s // 512            # 4 query chunks
    nc = bacc.Bacc("TRN2", target_bir_lowering=False, debug=False)

    x_d = nc.dram_tensor("x", [d, s], F16, kind="ExternalInput")
    wq_d = nc.dram_tensor("wq", [d, GW], F16, kind="ExternalInput")
    wk_d = nc.dram_tensor("wk", [d, GW], F16, kind="ExternalInput")
    wv_d = nc.dram_tensor("wv", [d, GW], F16, kind="ExternalInput")
    bq_d = nc.dram_tensor("bq", [128, 4], F32, kind="ExternalInput")
    bk_d = nc.dram_tensor("bk", [128, 4], F32, kind="ExternalInput")
    wo_d = nc.dram_tensor("wo", [GW, d], F16, kind="ExternalInput")
    out_d = nc.dram_tensor("out", [s, d], F16, kind="ExternalOutput")

    with TileContext(nc) as tc:
        with (
            tc.tile_pool(name="const", bufs=1) as cpool,
            tc.tile_pool(name="wpool", bufs=1) as wpool,
            tc.tile_pool(name="xt", bufs=n_kt) as xt_pool,
            tc.tile_pool(name="qk", bufs=2 * NP) as qk_pool,
            tc.tile_pool(name="vt", bufs=1) as vt_pool,
            tc.tile_pool(name="at", bufs=NP) as at_pool,
            tc.tile_pool(name="pex", bufs=6) as pex_pool,
            tc.tile_pool(name="rl", bufs=2) as rl_pool,
            tc.tile_pool(name="osb", bufs=2) as osb_pool,
            tc.tile_pool(name="ps", bufs=1, space="PSUM") as pp,
        ):
            # ---- loads: first half of x + wq first, so proj starts ASAP ----
            xT = [None] * n_kt
            w16 = {}

            def load_x(kts):
                for kt in kts:
                    xt = xt_pool.tile([128, s], F16, tag="xt", name=f"xT{kt}")
                    nc.sync.dma_start(
                        out=xt[:, :], in_=x_d[kt * 128:(kt + 1) * 128, :])
                    xT[kt] = xt

            def load_w(name, dram):
                for kt in range(n_kt):
                    wt = wpool.tile([128, GW], F16, tag="w", bufs=3 * n_kt,
                                    name=f"w_{name}{kt}")
                    nc.sync.dma_start(
                        out=wt[:, :], in_=dram[kt * 128:(kt + 1) * 128, :])
                    w16[(name, kt)] = wt

            load_x(range(4))
            load_w("q", wq_d)
            load_x(range(4, n_kt))
            load_w("k", wk_d)
            load_w("v", wv_d)

            bqt = cpool.tile([128, 4], F32, tag="bqt")
            bkt = cpool.tile([128, 4], F32, tag="bkt")
            nc.sync.dma_start(out=bqt[:, :], in_=bq_d[:, :])
            nc.sync.dma_start(out=bkt[:, :], in_=bk_d[:, :])

            wo_sb = []
            for hp in range(NP):
                wt = wpool.tile([128, d], F16, tag="wo", bufs=NP, name=f"wo{hp}")
                nc.sync.dma_start(
                    out=wt[:, :], in_=wo_d[hp * 128:(hp + 1) * 128, :])
                wo_sb.append(wt)

            # vt: [128 tok, (hp, tt, two, 65)] fp16, ones at col 64
            vt = vt_pool.tile([128, NP * n_tt * 130 + 128], F16, tag="vt")
            nc.vector.memset(vt[:, NP * n_tt * 130:], 0.0)
            vt4 = vt[:, 0:NP * n_tt * 130].rearrange(
                "p (hp t two c) -> p hp t two c", hp=NP, t=n_tt, two=2, c=65)
            nc.vector.memset(vt4[:, :, :, :, 64:65], 1.0)

            qT = [qk_pool.tile([128, s], F16, tag="qz", bufs=2 * NP,
                               name=f"qT{h}") for h in range(2 * NP)]
            kT = [qk_pool.tile([128, s], F16, tag="qk", name=f"kT{hp}")
                  for hp in range(NP)]
            for h in range(2 * NP):
                if h % 2 == 0:
                    nc.vector.memset(qT[h][64:128, :], 0.0)
                else:
                    nc.vector.memset(qT[h][0:64, :], 0.0)
            attnT = [at_pool.tile([128, s], F16, tag="at", name=f"aT{hp}")
                     for hp in range(NP)]

            # ---- Q/K projections (hp 0 first so attention can start) ----
            def proj_qk(hp):
                for name, bias, dst in (("q", bqt, None), ("k", bkt, kT[hp])):
                    p1 = pp.tile([128, 512], F32, tag="sta", bufs=2,
                                 name=f"pj1_{name}{hp}")
                    p2 = pp.tile([128, 512], F32, tag="stb", bufs=2,
                                 name=f"pj2_{name}{hp}")
                    p3 = pp.tile([128, 512], F32, tag="ps1", bufs=4,
                                 name=f"pj3_{name}{hp}")
                    p4 = pp.tile([128, 512], F32, tag="ps1", bufs=4,
                                 name=f"pj4_{name}{hp}")
                    accs = [p1[:, :], p2[:, :], p3[:, :], p4[:, :]]
                    for kt in range(n_kt):
                        lh = w16[(name, kt)][:, hp * 128:(hp + 1) * 128]
                        for tcn in range(4):
                            nc.tensor.matmul(
                                accs[tcn], lh,
                                xT[kt][:, tcn * 512:(tcn + 1) * 512],
                                start=(kt == 0), stop=(kt == n_kt - 1),
                            )
                    for tcn in range(4):
                        cs = slice(tcn * 512, (tcn + 1) * 512)
                        if name == "k":
                            nc.vector.tensor_scalar_add(
                                dst[:, cs], accs[tcn], bias[:, hp:hp + 1])
                        else:
                            nc.vector.tensor_scalar_add(
                                qT[2 * hp][0:64, cs], accs[tcn][0:64, :],
                                bias[0:64, hp:hp + 1])
                            nc.vector.tensor_scalar_add(
                                qT[2 * hp + 1][64:128, cs],
                                accs[tcn][64:128, :], bias[64:128, hp:hp + 1])

            proj_qk(0)

            # ---- V projection (natural layout, interleaved into vt) ----
            for tt in range(n_tt):
                pv = pp.tile([128, 512], F32, tag="ps1", bufs=4, name=f"pv{tt}")
                for kt in range(n_kt):
                    nc.tensor.matmul(
                        pv[:, :], xT[kt][:, tt * 128:(tt + 1) * 128],
                        w16[("v", kt)][:, :],
                        start=(kt == 0), stop=(kt == n_kt - 1),
                    )
                pv4 = pv.rearrange("p (hp two c) -> p hp two c",
                                   hp=NP, two=2, c=64)
                nc.vector.tensor_copy(vt4[:, :, tt, 0, 0:64], pv4[:, :, 0, :])
                nc.vector.tensor_copy(vt4[:, :, tt, 1, 0:64], pv4[:, :, 1, :])

            # ---- attention (proj of next pair interleaved between windows) ----
            for hp in range(NP):
                if hp + 1 < NP:
                    proj_qk(hp + 1)
                for jc in range(n_jc):
                    qs = slice(jc * 512, (jc + 1) * 512)
                    av0 = pp.tile([128, 512], F32, tag="ps1", bufs=4,
                                  name=f"av0_{hp}_{jc}")
                    av1 = pp.tile([128, 512], F32, tag="ps1", bufs=4,
                                  name=f"av1_{hp}_{jc}")
                    for kt in range(n_tt):
                        ks = slice(kt * 128, (kt + 1) * 128)
                        st0 = pp.tile([128, 512], F32, tag="sta", bufs=2,
                                      name=f"st0_{hp}_{jc}_{kt}")
                        st1 = pp.tile([128, 512], F32, tag="stb", bufs=2,
                                      name=f"st1_{hp}_{jc}_{kt}")
                        nc.tensor.matmul(
                            st0[:, :], kT[hp][:, ks], qT[2 * hp][:, qs],
                            start=True, stop=True,
                        )
                        nc.tensor.matmul(
                            st1[:, :], kT[hp][:, ks],
                            qT[2 * hp + 1][:, qs],
                            start=True, stop=True,
                        )
                        pex0 = pex_pool.tile([128, 512], F16, tag="pexa",
                                             bufs=4, name=f"pex0_{hp}_{jc}_{kt}")
                        pex1 = pex_pool.tile([128, 512], F16, tag="pexb",
                                             bufs=4, name=f"pex1_{hp}_{jc}_{kt}")
                        nc.scalar.activation(
                            pex0[:, :], st0[:, :], AF.Exp, scale=EXP_SCALE)
                        nc.vector.tensor_scalar_add(
                            pex1[:, :].bitcast(I16), st1[:, :], BCONST)
                        vbase = (hp * n_tt + kt) * 130
                        nc.tensor.matmul(
                            av0[:, :], vt[:, vbase:vbase + 128],
                            pex0[:, :],
                            start=(kt == 0), stop=(kt == n_tt - 1),
                        )
                        nc.tensor.matmul(
                            av1[:, :], vt[:, vbase + 65:vbase + 193],
                            pex1[:, :],
                            start=(kt == 0), stop=(kt == n_tt - 1),
                        )
                    # drain: r rows -> 1/r -> broadcast -> scaled attnT copy
                    rline = rl_pool.tile([1, 1024], F32, tag="rl",
                                         name=f"rline_{hp}_{jc}")
                    rrec = rl_pool.tile([1, 1024], F32, tag="rr",
                                        name=f"rrec_{hp}_{jc}")
                    rb = rl_pool.tile([64, 1024], F32, tag="rb",
                                      name=f"rb_{hp}_{jc}")
                    nc.scalar.copy(rline[0:1, 0:512], av0[64:65, :])
                    nc.scalar.copy(rline[0:1, 512:1024], av1[64:65, :])
                    nc.vector.reciprocal_approx_fast(rrec[0:1, :],
                                                     rline[0:1, :])
                    nc.gpsimd.partition_broadcast(rb[:, :], rrec[0:1, :])
                    nc.vector.tensor_mul(attnT[hp][0:64, qs], av0[0:64, :],
                                         rb[:, 0:512])
                    nc.vector.tensor_mul(attnT[hp][64:128, qs], av1[0:64, :],
                                         rb[:, 512:1024])

            # ---- output projection: accumulate 4 pairs in PSUM ----
            for tt in range(n_tt):
                ts_ = slice(tt * 128, (tt + 1) * 128)
                po0 = pp.tile([128, 512], F32, tag="ps1", bufs=4, name=f"po0_{tt}")
                po1 = pp.tile([128, 512], F32, tag="ps1", bufs=4, name=f"po1_{tt}")
                for hp in range(NP):
                    nc.tensor.matmul(
                        po0[:, :], attnT[hp][:, ts_], wo_sb[hp][:, 0:512],
                        start=(hp == 0), stop=(hp == NP - 1),
                    )
                    nc.tensor.matmul(
                        po1[:, :], attnT[hp][:, ts_], wo_sb[hp][:, 512:1024],
                        start=(hp == 0), stop=(hp == NP - 1),
                    )
                osb = osb_pool.tile([128, 1024], F16, tag="osb",
                                    name=f"osb{tt}")
                nc.vector.tensor_copy(osb[:, 0:512], po0[:, :])
                nc.scalar.copy(osb[:, 512:1024], po1[:, :])
                nc.sync.dma_start(out=out_d[ts_, :], in_=osb[:, :])

    nc.compile()
    return nc


def _get_nc():
    if "nc" not in _CACHE:
        _CACHE["nc"] = build_nc()
    return _CACHE["nc"]


def make_in_maps(x, w_q, b_q, w_k, b_k, w_v, w_o):
    sF = np.float32(np.sqrt(FSC))
    xT16 = [np.ascontiguousarray(np.asarray(x[b], np.float16).T)
            for b in range(x.shape[0])]
    wq16 = np.asarray(np.asarray(w_q, np.float32) * sF, np.float16)
    wk16 = np.asarray(np.asarray(w_k, np.float32) * sF, np.float16)
    wv16 = np.asarray(w_v, np.float16)
    wo16 = np.asarray(w_o, np.float16)
    bq32 = np.asarray(b_q, np.float32) * sF
    bk32 = np.asarray(b_k, np.float32) * sF

    in_maps = []
    for i in range(NCORES):
        b, g = i // 2, i % 2
        gs = slice(g * GW, (g + 1) * GW)
        in_maps.append({
            "x": xT16[b],
            "wq": np.ascontiguousarray(wq16[:, gs]),
            "wk": np.ascontiguousarray(wk16[:, gs]),
            "wv": np.ascontiguousarray(wv16[:, gs]),
            "bq": np.ascontiguousarray(bq32[gs].reshape(4, 128).T),
            "bk": np.ascontiguousarray(bk32[gs].reshape(4, 128).T),
            "wo": np.ascontiguousarray(wo16[gs, :]),
        })
    return in_maps


def kernel(x, w_q, b_q, w_k, b_k, w_v, b_v, w_o, b_o, _trace=False):
    x = np.asarray(x, dtype=np.float32)
    nc = _get_nc()
    in_maps = make_in_maps(x, w_q, b_q, w_k, b_k, w_v, w_o)
    kw = {}
    if _trace:
        import tempfile
        kw = dict(trace=True, trace_cores=list(range(NCORES)),
                  tmpdir=tempfile.mkdtemp(prefix="mha_trace_"))
    res = bass_utils.run_bass_kernel_spmd(
        nc, in_maps, core_ids=list(range(NCORES)), **kw
    )
    out = np.zeros(x.shape, dtype=np.float32)
    for b in range(x.shape[0]):
        out[b] = (np.asarray(res.results[2 * b]["out"], dtype=np.float32)
                  + np.asarray(res.results[2 * b + 1]["out"],
                               dtype=np.float32))
    out += np.asarray(b_o, dtype=np.float32)[None, None, :]
    out += (np.asarray(b_v, dtype=np.float32)
            @ np.asarray(w_o, dtype=np.float32))[None, None, :]
    if _trace:
        return out, res
    return out


# revision 6
# speedup vs baseline: 1.0091x; 1.0064x over previous
"""Multi-head attention on 8 Trainium2 NeuronCores — batch x head sharding.

Core i handles batch b=i//2 and head-group g=i%2 (8 heads, 512 proj dims).
Host sums the 2 partial outputs per batch and adds b_o + b_v @ w_o
(sum_k softmax = 1 makes the v-bias contribution exactly b_v @ w_o).

Design notes (what made this fast — 550us baseline -> ~400us):
- Every matmul is full-K=128 with a 128-column fp16 lhsT and no
  tile_position. In that shape walrus/PE hide LDWEIGHTS completely
  (216 ns/MM sustained); row-tiled tile_position pairs do NOT overlap
  under this toolchain and their weight loads serialize, measured slower.
- Scores: lhsT = kT pair tile (shared by both heads -> weight reuse);
  the per-head moving operand qT is zero-padded ([q_h; 0] / [0; q_h]) so
  the other head's rows contribute nothing.
- AV: vt layout [V_h0 | 1 | V_h1 | 1] per 130 cols; lhsT slices are
  widened to 128 cols so both heads read [V_h | 1 | junk]; the junk rows
  land in unused PSUM partitions, the ones-column yields the softmax
  denominator r at psum row 64 for both heads.
- Softmax exp is split across two engines per kt: head-even as exact ACT
  exp, head-odd as a one-op Schraudolph on the DVE: scores are pre-scaled
  by F = 128*log2(e) (folded sqrt(F) into w_q AND w_k), so
  exp(s/8) = 2^(s'/1024) = bitcast_fp16(int16(s' + 15360)).
- 1/r via reciprocal_approx_fast + gpsimd partition_broadcast; the
  PSUM->SBUF attnT copy is fused with the 1/r scaling (tensor_mul with a
  partition-shifted output for the odd head).
- Output projection accumulates all 4 head-pairs in PSUM, one copy out.
- Fine-grained single-bank PSUM tiles (per-head st/pexp) decouple the two
  exp engine chains; keeping PE duty >85 percent also keeps the HAM clock
  gate at 2.4 GHz (coarse tiles measured 30 percent slower from K=4/8
  throttling).
"""

import numpy as np

import concourse.bacc as bacc
import concourse.mybir as mybir
from concourse.tile import TileContext
from concourse import bass_utils

dt = mybir.dt
F32 = dt.float32
F16 = dt.float16
I16 = dt.int16
AF = mybir.ActivationFunctionType
ALU = mybir.AluOpType

B, S, D = 4, 2048, 1024
H, DH = 16, 64
NCORES = 8
HC = 8                      # heads per core
GW = HC * DH                # 512 proj cols per core
NP = HC // 2                # head pairs = 4

FSC = 128.0 * np.log2(np.e)          # score pre-scale (folded into wq,wk)
EXP_SCALE = float(1.0 / (8.0 * FSC))  # ACT exp scale on pre-scaled scores
BCONST = 15360.0                      # schraudolph bias (fp16 exponent 15)

_CACHE = {}


def build_nc(s=S, d=D):
    n_kt = d // 128            # 8 contraction tiles
    n_tt = s // 128            # 16 token tiles
    n_jc = s // 512            # 4 query chunks
    nc = bacc.Bacc("TRN2", target_bir_lowering=False, debug=False)

    x_d = nc.dram_tensor("x", [d, s], F16, kind="ExternalInput")
    wq_d = nc.dram_tensor("wq", [d, GW], F16, kind="ExternalInput")
    wk_d = nc.dram_tensor("wk", [d, GW], F16, kind="ExternalInput")
    wv_d = nc.dram_tensor("wv", [d, GW], F16, kind="ExternalInput")
    bq_d = nc.dram_tensor("bq", [128, 4], F32, kind="ExternalInput")
    bk_d = nc.dram_tensor("bk", [128, 4], F32, kind="ExternalInput")
    wo_d = nc.dram_tensor("wo", [GW, d], F16, kind="ExternalInput")
    out_d = nc.dram_tensor("out", [s, d], F16, kind="ExternalOutput")

    with TileContext(nc) as tc:
        with (
            tc.tile_pool(name="const", bufs=1) as cpool,
            tc.tile_pool(name="wpool", bufs=1) as wpool,
            tc.tile_pool(name="xt", bufs=n_kt) as xt_pool,
            tc.tile_pool(name="qk", bufs=2 * NP) as qk_pool,
            tc.tile_pool(name="vt", bufs=1) as vt_pool,
            tc.tile_pool(name="at", bufs=NP) as at_pool,
            tc.tile_pool(name="pex", bufs=6) as pex_pool,
            tc.tile_pool(name="rl", bufs=2) as rl_pool,
            tc.tile_pool(name="osb", bufs=2) as osb_pool,
            tc.tile_pool(name="ps", bufs=1, space="PSUM") as pp,
        ):
            # ---- loads: first half of x + wq first, so proj starts ASAP ----
            xT = [None] * n_kt
            w16 = {}

            def load_x(kts):
                for kt in kts:
                    xt = xt_pool.tile([128, s], F16, tag="xt", name=f"xT{kt}")
                    nc.sync.dma_start(
                        out=xt[:, :], in_=x_d[kt * 128:(kt + 1) * 128, :])
                    xT[kt] = xt

            def load_w(name, dram):
                for kt in range(n_kt):
                    wt = wpool.tile([128, GW], F16, tag="w", bufs=3 * n_kt,
                                    name=f"w_{name}{kt}")
                    nc.sync.dma_start(
                        out=wt[:, :], in_=dram[kt * 128:(kt + 1) * 128, :])
                    w16[(name, kt)] = wt

            load_x(range(4))
            load_w("q", wq_d)
            load_x(range(4, n_kt))
            load_w("k", wk_d)
            load_w("v", wv_d)

            bqt = cpool.tile([128, 4], F32, tag="bqt")
            bkt = cpool.tile([128, 4], F32, tag="bkt")
            nc.sync.dma_start(out=bqt[:, :], in_=bq_d[:, :])
            nc.sync.dma_start(out=bkt[:, :], in_=bk_d[:, :])

            wo_sb = []
            for hp in range(NP):
                wt = wpool.tile([128, d], F16, tag="wo", bufs=NP, name=f"wo{hp}")
                nc.sync.dma_start(
                    out=wt[:, :], in_=wo_d[hp * 128:(hp + 1) * 128, :])
                wo_sb.append(wt)

            # vt: [128 tok, (hp, tt, two, 65)] fp16, ones at col 64
            vt = vt_pool.tile([128, NP * n_tt * 130 + 128], F16, tag="vt")
            nc.vector.memset(vt[:, NP * n_tt * 130:], 0.0)
            vt4 = vt[:, 0:NP * n_tt * 130].rearrange(
                "p (hp t two c) -> p hp t two c", hp=NP, t=n_tt, two=2, c=65)
            nc.vector.memset(vt4[:, :, :, :, 64:65], 1.0)

            qT = [qk_pool.tile([128, s], F16, tag="qz", bufs=2 * NP,
                               name=f"qT{h}") for h in range(2 * NP)]
            kT = [qk_pool.tile([128, s], F16, tag="qk", name=f"kT{hp}")
                  for hp in range(NP)]
            for h in range(2 * NP):
                if h % 2 == 0:
                    nc.vector.memset(qT[h][64:128, :], 0.0)
                else:
                    nc.vector.memset(qT[h][0:64, :], 0.0)
            attnT = [at_pool.tile([128, s], F16, tag="at", name=f"aT{hp}")
                     for hp in range(NP)]

            # ---- Q/K projections (hp 0 first so attention can start) ----
            def proj_qk(hp):
                for name, bias, dst in (("q", bqt, None), ("k", bkt, kT[hp])):
                    p1 = pp.tile([128, 512], F32, tag="sta", bufs=2,
                                 name=f"pj1_{name}{hp}")
                    p2 = pp.tile([128, 512], F32, tag="stb", bufs=2,
                                 name=f"pj2_{name}{hp}")
                    p3 = pp.tile([128, 512], F32, tag="ps1", bufs=4,
                                 name=f"pj3_{name}{hp}")
                    p4 = pp.tile([128, 512], F32, tag="ps1", bufs=4,
                                 name=f"pj4_{name}{hp}")
                    accs = [p1[:, :], p2[:, :], p3[:, :], p4[:, :]]
                    for kt in range(n_kt):
                        lh = w16[(name, kt)][:, hp * 128:(hp + 1) * 128]
                        for tcn in range(4):
                            nc.tensor.matmul(
                                accs[tcn], lh,
                                xT[kt][:, tcn * 512:(tcn + 1) * 512],
                                start=(kt == 0), stop=(kt == n_kt - 1),
                            )
                    for tcn in range(4):
                        cs = slice(tcn * 512, (tcn + 1) * 512)
                        if name == "k":
                            nc.vector.tensor_scalar_add(
                                dst[:, cs], accs[tcn], bias[:, hp:hp + 1])
                        else:
                            nc.vector.tensor_scalar_add(
                                qT[2 * hp][0:64, cs], accs[tcn][0:64, :],
                                bias[0:64, hp:hp + 1])
                            nc.vector.tensor_scalar_add(
                                qT[2 * hp + 1][64:128, cs],
                                accs[tcn][64:128, :], bias[64:128, hp:hp + 1])

            proj_qk(0)

            # ---- V projection (natural layout, interleaved into vt) ----
            for tt in range(n_tt):
                pv = pp.tile([128, 512], F32, tag="ps1", bufs=4, name=f"pv{tt}")
                for kt in range(n_kt):
                    nc.tensor.matmul(
                        pv[:, :], xT[kt][:, tt * 128:(tt + 1) * 128],
                        w16[("v", kt)][:, :],
                        start=(kt == 0), stop=(kt == n_kt - 1),
                    )
                pv4 = pv.rearrange("p (hp two c) -> p hp two c",
                                   hp=NP, two=2, c=64)
                nc.vector.tensor_copy(vt4[:, :, tt, 0, 0:64], pv4[:, :, 0, :])
                nc.vector.tensor_copy(vt4[:, :, tt, 1, 0:64], pv4[:, :, 1, :])

            # ---- attention (proj of next pair interleaved between windows) ----
            for hp in range(NP):
                if hp + 1 < NP:
                    proj_qk(hp + 1)
                for jc in range(n_jc):
                    qs = slice(jc * 512, (jc + 1) * 512)
                    av0 = pp.tile([128, 512], F32, tag="ps1", bufs=4,
                                  name=f"av0_{hp}_{jc}")
                    av1 = pp.tile([128, 512], F32, tag="ps1", bufs=4,
                                  name=f"av1_{hp}_{jc}")
                    for kt in range(n_tt):
                        ks = slice(kt * 128, (kt + 1) * 128)
                        st0 = pp.tile([128, 512], F32, tag="sta", bufs=2,
                                      name=f"st0_{hp}_{jc}_{kt}")
                        st1 = pp.tile([128, 512], F32, tag="stb", bufs=2,
                                      name=f"st1_{hp}_{jc}_{kt}")
                        nc.tensor.matmul(
                            st0[:, :], kT[hp][:, ks], qT[2 * hp][:, qs],
                            start=True, stop=True,
                        )
                        nc.tensor.matmul(
                            st1[:, :], kT[hp][:, ks],
                            qT[2 * hp + 1][:, qs],
                            start=True, stop=True,
                        )
                        pex0 = pex_pool.tile([128, 512], F16, tag="pexa",
                                             bufs=6, name=f"pex0_{hp}_{jc}_{kt}")
                        pex1 = pex_pool.tile([128, 512], F16, tag="pexb",
                                             bufs=6, name=f"pex1_{hp}_{jc}_{kt}")
                        nc.scalar.activation(
                            pex0[:, :], st0[:, :], AF.Exp, scale=EXP_SCALE)
                        nc.vector.tensor_scalar_add(
                            pex1[:, :].bitcast(I16), st1[:, :], BCONST)
                        vbase = (hp * n_tt + kt) * 130
                        nc.tensor.matmul(
                            av0[:, :], vt[:, vbase:vbase + 128],
                            pex0[:, :],
                            start=(kt == 0), stop=(kt == n_tt - 1),
                        )
                        nc.tensor.matmul(
                            av1[:, :], vt[:, vbase + 65:vbase + 193],
                            pex1[:, :],
                            start=(kt == 0), stop=(kt == n_tt - 1),
                        )
                    # drain: r rows -> 1/r -> broadcast -> scaled attnT copy
                    rline = rl_pool.tile([1, 1024], F32, tag="rl",
                                         name=f"rline_{hp}_{jc}")
                    rrec = rl_pool.tile([1, 1024], F32, tag="rr",
                                        name=f"rrec_{hp}_{jc}")
                    rb = rl_pool.tile([64, 1024], F32, tag="rb",
                                      name=f"rb_{hp}_{jc}")
                    nc.scalar.copy(rline[0:1, 0:512], av0[64:65, :])
                    nc.scalar.copy(rline[0:1, 512:1024], av1[64:65, :])
                    nc.vector.reciprocal_approx_fast(rrec[0:1, :],
                                                     rline[0:1, :])
                    nc.gpsimd.partition_broadcast(rb[:, :], rrec[0:1, :])
                    nc.vector.tensor_mul(attnT[hp][0:64, qs], av0[0:64, :],
                                         rb[:, 0:512])
                    nc.vector.tensor_mul(attnT[hp][64:128, qs], av1[0:64, :],
                                         rb[:, 512:1024])

            # ---- output projection: accumulate 4 pairs in PSUM ----
            for tt in range(n_tt):
                ts_ = slice(tt * 128, (tt + 1) * 128)
                po0 = pp.tile([128, 512], F32, tag="ps1", bufs=4, name=f"po0_{tt}")
                po1 = pp.tile([128, 512], F32, tag="ps1", bufs=4, name=f"po1_{tt}")
                for hp in range(NP):
                    nc.tensor.matmul(
                        po0[:, :], attnT[hp][:, ts_], wo_sb[hp][:, 0:512],
                        start=(hp == 0), stop=(hp == NP - 1),
                    )
                    nc.tensor.matmul(
                        po1[:, :], attnT[hp][:, ts_], wo_sb[hp][:, 512:1024],
                        start=(hp == 0), stop=(hp == NP - 1),
                    )
                osb = osb_pool.tile([128, 1024], F16, tag="osb",
                                    name=f"osb{tt}")
                nc.vector.tensor_copy(osb[:, 0:512], po0[:, :])
                nc.scalar.copy(osb[:, 512:1024], po1[:, :])
                nc.sync.dma_start(out=out_d[ts_, :], in_=osb[:, :])

    nc.compile()
    return nc


def _get_nc():
    if "nc" not in _CACHE:
        _CACHE["nc"] = build_nc()
    return _CACHE["nc"]


def make_in_maps(x, w_q, b_q, w_k, b_k, w_v, w_o):
    sF = np.float32(np.sqrt(FSC))
    xT16 = [np.ascontiguousarray(np.asarray(x[b], np.float16).T)
            for b in range(x.shape[0])]
    wq16 = np.asarray(np.asarray(w_q, np.float32) * sF, np.float16)
    wk16 = np.asarray(np.asarray(w_k, np.float32) * sF, np.float16)
    wv16 = np.asarray(w_v, np.float16)
    wo16 = np.asarray(w_o, np.float16)
    bq32 = np.asarray(b_q, np.float32) * sF
    bk32 = np.asarray(b_k, np.float32) * sF

    in_maps = []
    for i in range(NCORES):
        b, g = i // 2, i % 2
        gs = slice(g * GW, (g + 1) * GW)
        in_maps.append({
            "x": xT16[b],
            "wq": np.ascontiguousarray(wq16[:, gs]),
            "wk": np.ascontiguousarray(wk16[:, gs]),
            "wv": np.ascontiguousarray(wv16[:, gs]),
            "bq": np.ascontiguousarray(bq32[gs].reshape(4, 128).T),
            "bk": np.ascontiguousarray(bk32[gs].reshape(4, 128).T),
            "wo": np.ascontiguousarray(wo16[gs, :]),
        })
    return in_maps


def kernel(x, w_q, b_q, w_k, b_k, w_v, b_v, w_o, b_o, _trace=False):
    x = np.asarray(x, dtype=np.float32)
    nc = _get_nc()
    in_maps = make_in_maps(x, w_q, b_q, w_k, b_k, w_v, w_o)
    kw = {}
    if _trace:
        import tempfile
        kw = dict(trace=True, trace_cores=list(range(NCORES)),
                  tmpdir=tempfile.mkdtemp(prefix="mha_trace_"))
    res = bass_utils.run_bass_kernel_spmd(
        nc, in_maps, core_ids=list(range(NCORES)), **kw
    )
    out = np.zeros(x.shape, dtype=np.float32)
    for b in range(x.shape[0]):
        out[b] = (np.asarray(res.results[2 * b]["out"], dtype=np.float32)
                  + np.asarray(res.results[2 * b + 1]["out"],
                               dtype=np.float32))
    out += np.asarray(b_o, dtype=np.float32)[None, None, :]
    out += (np.asarray(b_v, dtype=np.float32)
            @ np.asarray(w_o, dtype=np.float32))[None, None, :]
    if _trace:
        return out, res
    return out
